# revision 1
# baseline (speedup 1.0000x reference)
"""Trainium2 Bass kernel for a biased transformer encoder layer.

Full (unsharded) inputs -> full output. Internally shards data-parallel over
batch B across 8 NeuronCores (one batch element per core). The bias tensor
(256MB) dominates memory traffic; it is host-transposed so the attention
works entirely in "transposed scores" layout (scores^T[t, s]) and no large
on-chip transpose is ever needed.
"""

import numpy as np
import ml_dtypes

import concourse.mybir as mybir
import concourse.tile as tile
from concourse import bacc
from concourse.bass_utils import run_bass_kernel_spmd

# ---- problem constants (hardcoded per contract) ----
S = 1024
B = 8
D = 256
H = 8
HD = D // H          # 32
DFF = 1024
EPS = 1e-5
N_CORES = 8

F32 = mybir.dt.float32
F32R = mybir.dt.float32r
BF16 = mybir.dt.bfloat16
bf16 = ml_dtypes.bfloat16

_CACHE = {}


def _install_axon_hooks_shim():
    """Make `trace=True` degrade gracefully if antenv.axon_hooks is missing."""
    import sys, types
    try:
        import antenv  # noqa
    except ImportError:
        return
    if "antenv.axon_hooks" in sys.modules:
        return
    try:
        import antenv.axon_hooks  # noqa
    except ImportError:
        import antenv
        mod = types.ModuleType("antenv.axon_hooks")
        _hook = [None]
        mod.set_axon_ntff_profile_hook = lambda h: _hook.__setitem__(0, h)
        mod.get_axon_ntff_profile_hook = lambda: _hook[0]
        sys.modules["antenv.axon_hooks"] = mod
        antenv.axon_hooks = mod


def _build(flags):
    """Build the Bass program (shared by all 8 cores, SPMD)."""
    (aff1, aff2, has_bqk, has_bo, has_b1, has_b2) = flags
    nc = bacc.Bacc("TRN2", debug=False, num_devices=N_CORES, enable_asserts=True)

    # ---- DRAM tensors (per-core inputs) ----
    src_d = nc.dram_tensor("src", [S, D], F32, kind="ExternalInput")
    expbT_d = nc.dram_tensor("expbT", [H, S, S], BF16, kind="ExternalInput")
    identb_d = nc.dram_tensor("identb", [128, 128], BF16, kind="ExternalInput")
    wqkT_d = nc.dram_tensor("wqkT", [D, 2 * D], BF16, kind="ExternalInput")
    wvxT_d = nc.dram_tensor("wvxT", [D, 2 * D], BF16, kind="ExternalInput")
    bvx_d = nc.dram_tensor("bvx", [128, 2 * D], F32, kind="ExternalInput")
    woT_d = nc.dram_tensor("woT", [D, D], BF16, kind="ExternalInput")
    w1T_d = nc.dram_tensor("w1T", [D, DFF], BF16, kind="ExternalInput")
    w2T_d = nc.dram_tensor("w2T", [DFF, D], BF16, kind="ExternalInput")
    # optional per-partition bias columns / broadcast tiles
    if has_bqk:
        bqk_d = nc.dram_tensor("bqk", [128, 4], F32, kind="ExternalInput")
    if aff1:
        g1b_d = nc.dram_tensor("g1b", [128, D], F32, kind="ExternalInput")
        be1b_d = nc.dram_tensor("be1b", [128, D], F32, kind="ExternalInput")
    if aff2:
        g2b_d = nc.dram_tensor("g2b", [128, D], F32, kind="ExternalInput")
        be2b_d = nc.dram_tensor("be2b", [128, D], F32, kind="ExternalInput")
    if has_bo:
        bob_d = nc.dram_tensor("bob", [128, D], F32, kind="ExternalInput")
    if has_b1:
        b1c_d = nc.dram_tensor("b1c", [128, DFF // 128], F32, kind="ExternalInput")
    if has_b2:
        b2c_d = nc.dram_tensor("b2c", [128, D // 128], F32, kind="ExternalInput")
    out_d = nc.dram_tensor("out", [S, D], F32, kind="ExternalOutput")
    import os as _os
    _DBG = bool(_os.environ.get("KERNEL_DEBUG"))
    if _DBG:
        dbg_r8a = nc.dram_tensor("dbg_r8a", [8, S], F32, kind="ExternalOutput")
        dbg_r8r = nc.dram_tensor("dbg_r8r", [8, S], F32, kind="ExternalOutput")
        dbg_ctxU = nc.dram_tensor("dbg_ctxU", [2, 128, S], F32, kind="ExternalOutput")
        dbg_ctxT = nc.dram_tensor("dbg_ctxT", [2, 128, S], F32, kind="ExternalOutput")
        dbg_R = nc.dram_tensor("dbg_R", [2, 128, S], F32, kind="ExternalOutput")

    NT = S // 128  # 8 s-tiles / t-tiles

    with tile.TileContext(nc, trace_sim=True) as tc:
        with tc.tile_pool(name="persist", bufs=1) as pp:
            # ---- load weights/constants ----
            identb = pp.tile([128, 128], BF16, tag="identb", name="identb")
            nc.sync.dma_start(identb[:], identb_d.ap())
            eps_t = pp.tile([128, 1], F32, tag="eps_t", name="eps_t")
            nc.gpsimd.memset(eps_t[:], EPS)
            wqkT = [pp.tile([128, 2 * D], BF16, tag=f"wqkT{k}", name=f"wqkT{k}") for k in range(2)]
            wvxT = [pp.tile([128, 2 * D], BF16, tag=f"wvxT{k}", name=f"wvxT{k}") for k in range(2)]
            woT = [pp.tile([128, D], BF16, tag=f"woT{k}", name=f"woT{k}") for k in range(2)]
            w1T = [pp.tile([128, DFF], BF16, tag=f"w1T{k}", name=f"w1T{k}") for k in range(2)]
            w2T = [pp.tile([128, D], BF16, tag=f"w2T{k}", name=f"w2T{k}") for k in range(8)]
            for k in range(2):
                nc.sync.dma_start(wqkT[k][:], wqkT_d.ap()[128 * k:128 * (k + 1), :])
                nc.sync.dma_start(wvxT[k][:], wvxT_d.ap()[128 * k:128 * (k + 1), :])
                nc.sync.dma_start(woT[k][:], woT_d.ap()[128 * k:128 * (k + 1), :])
                nc.sync.dma_start(w1T[k][:], w1T_d.ap()[128 * k:128 * (k + 1), :])
            for k in range(8):
                nc.sync.dma_start(w2T[k][:], w2T_d.ap()[128 * k:128 * (k + 1), :])
            bvx = pp.tile([128, 2 * D], F32, tag="bvx", name="bvx")
            nc.sync.dma_start(bvx[:], bvx_d.ap())
            if has_bqk:
                bqk = pp.tile([128, 4], F32, tag="bqk", name="bqk")
                nc.sync.dma_start(bqk[:], bqk_d.ap())
            if aff1:
                g1b = pp.tile([128, D], F32, tag="g1b", name="g1b")
                be1b = pp.tile([128, D], F32, tag="be1b", name="be1b")
                nc.sync.dma_start(g1b[:], g1b_d.ap())
                nc.sync.dma_start(be1b[:], be1b_d.ap())
            if aff2:
                g2b = pp.tile([128, D], F32, tag="g2b", name="g2b")
                be2b = pp.tile([128, D], F32, tag="be2b", name="be2b")
                nc.sync.dma_start(g2b[:], g2b_d.ap())
                nc.sync.dma_start(be2b[:], be2b_d.ap())
            if has_bo:
                bob = pp.tile([128, D], F32, tag="bob", name="bob")
                nc.sync.dma_start(bob[:], bob_d.ap())
            if has_b1:
                b1c = pp.tile([128, DFF // 128], F32, tag="b1c", name="b1c")
                nc.sync.dma_start(b1c[:], b1c_d.ap())
            if has_b2:
                b2c = pp.tile([128, D // 128], F32, tag="b2c", name="b2c")
                nc.sync.dma_start(b2c[:], b2c_d.ap())

            # persistent activations
            xn = [pp.tile([128, D], F32, tag=f"xn{i}", name=f"xn{i}") for i in range(NT)]
            x_res = xn  # residual source (replaced if aff1)
            if aff1:
                x_res = [pp.tile([128, D], F32, tag=f"xr{i}", name=f"xr{i}") for i in range(NT)]
            xnT = [pp.tile([128, S], BF16, tag=f"xnT{k}", name=f"xnT{k}") for k in range(2)]
            qT = [pp.tile([128, S], BF16, tag=f"qT{k}", name=f"qT{k}") for k in range(2)]
            kT = [pp.tile([128, S], BF16, tag=f"kT{k}", name=f"kT{k}") for k in range(2)]
            vx = [pp.tile([128, 2 * D], BF16, tag=f"vx{i}", name=f"vx{i}") for i in range(NT)]
            ctxT = [pp.tile([128, S], BF16, tag=f"ctxT{k}", name=f"ctxT{k}") for k in range(2)]
            ctxU = [pp.tile([128, S], BF16, tag=f"ctxU{k}", name=f"ctxU{k}") for k in range(2)]
            Rden = [pp.tile([128, S], F32, tag=f"Rden{k}", name=f"Rden{k}") for k in range(2)]
            Rrec = [pp.tile([128, S], F32, tag=f"Rrec{k}", name=f"Rrec{k}") for k in range(2)]
            yn = [pp.tile([128, D], F32, tag=f"yn{i}", name=f"yn{i}") for i in range(NT)]
            y_res = yn
            if aff2:
                y_res = [pp.tile([128, D], F32, tag=f"yr{i}", name=f"yr{i}") for i in range(NT)]
            ynT = [pp.tile([128, S], BF16, tag=f"ynT{k}", name=f"ynT{k}") for k in range(2)]
            f1T = [pp.tile([128, S], BF16, tag=f"f1T{m}", name=f"f1T{m}") for m in range(8)]
            f2T = [pp.tile([128, S], BF16, tag=f"f2T{m}", name=f"f2T{m}") for m in range(2)]

            def layer_norm(src_tiles, out_tiles, work, tmp_bf=None):
                """out = (src - mean)/sqrt(var+eps) per row; optional bf16 copy."""
                for i in range(NT):
                    st = src_tiles[i]
                    stats = work.tile([128, 6], F32, tag="lnstats", name="lnstats")
                    nc.vector.bn_stats(stats[:], st[:])
                    aggr = work.tile([128, 2], F32, tag="lnaggr", name="lnaggr")
                    nc.vector.bn_aggr(aggr[:], stats[:])
                    sd = work.tile([128, 1], F32, tag="lnsd", name="lnsd")
                    nc.scalar.activation(sd[:], aggr[:, 1:2],
                                         mybir.ActivationFunctionType.Sqrt, bias=eps_t[:, 0:1])
                    rs = work.tile([128, 1], F32, tag="lnrs", name="lnrs")
                    nc.vector.reciprocal(rs[:], sd[:])
                    nc.vector.tensor_scalar(
                        out_tiles[i][:], st[:], aggr[:, 0:1], rs[:],
                        mybir.AluOpType.subtract, mybir.AluOpType.mult)
                    if tmp_bf is not None:
                        nc.vector.tensor_copy(tmp_bf[i][:], out_tiles[i][:])

            def transpose_to(src_bf_tiles, dstT, psum_pool):
                """[S, D]-tiled bf16 (NT tiles of [128, D]) -> dstT (2 tiles [128, S])."""
                for j in range(2):           # d block
                    for i0 in range(0, NT, 4):
                        tp = psum_pool.tile([128, 512], BF16, tag="tp", name="tp")
                        for di in range(4):
                            i = i0 + di
                            nc.tensor.transpose(
                                tp[:, 128 * di:128 * (di + 1)],
                                src_bf_tiles[i][:, 128 * j:128 * (j + 1)],
                                identb[:])
                        nc.vector.tensor_copy(
                            dstT[j][:, 128 * i0:128 * (i0 + 4)], tp[:])

            # ================= Phase 1: LN1, qkT, v_ext =================
            with tc.tile_pool(name="work1", bufs=4) as wk, \
                 tc.tile_pool(name="ps1", bufs=2, space="PSUM") as ps1:
                xbf = [pp.tile([128, D], BF16, tag=f"xbf{i}", name=f"xbf{i}") for i in range(NT)]
                for i in range(NT):
                    st = wk.tile([128, D], F32, tag="srct", name="srct")
                    nc.sync.dma_start(st[:], src_d.ap()[128 * i:128 * (i + 1), :])
                    stats = wk.tile([128, 6], F32, tag="lnstats", name="lnstats")
                    nc.vector.bn_stats(stats[:], st[:])
                    aggr = wk.tile([128, 2], F32, tag="lnaggr", name="lnaggr")
                    nc.vector.bn_aggr(aggr[:], stats[:])
                    sd = wk.tile([128, 1], F32, tag="lnsd", name="lnsd")
                    nc.scalar.activation(sd[:], aggr[:, 1:2],
                                         mybir.ActivationFunctionType.Sqrt, bias=eps_t[:, 0:1])
                    rs = wk.tile([128, 1], F32, tag="lnrs", name="lnrs")
                    nc.vector.reciprocal(rs[:], sd[:])
                    nc.vector.tensor_scalar(
                        xn[i][:], st[:], aggr[:, 0:1], rs[:],
                        mybir.AluOpType.subtract, mybir.AluOpType.mult)
                    nc.gpsimd.tensor_copy(xbf[i][:], xn[i][:])
                    if aff1:
                        tmp = wk.tile([128, D], F32, tag="afftmp", name="afftmp")
                        nc.vector.tensor_tensor(tmp[:], xn[i][:], g1b[:],
                                                mybir.AluOpType.mult)
                        nc.vector.tensor_tensor(x_res[i][:], tmp[:], be1b[:],
                                                mybir.AluOpType.add)
                    # transpose this s-tile into xnT columns (both d-blocks)
                    tp = ps1.tile([128, 256], BF16, tag="tp", name="tp")
                    for j in range(2):
                        nc.tensor.transpose(
                            tp[:, 128 * j:128 * (j + 1)],
                            xbf[i][:, 128 * j:128 * (j + 1)], identb[:])
                    for j in range(2):
                        nc.vector.tensor_copy(
                            xnT[j][:, 128 * i:128 * (i + 1)],
                            tp[:, 128 * j:128 * (j + 1)])
                    # v_ext for this tile
                    pv = ps1.tile([128, 512], F32, tag="pv", name="pv")
                    for k in range(2):
                        nc.tensor.matmul(
                            pv[:],
                            xnT[k][:, 128 * i:128 * (i + 1)],
                            wvxT[k][:],
                            start=(k == 0), stop=(k == 1))
                    nc.vector.tensor_tensor(vx[i][:], pv[:], bvx[:],
                                            mybir.AluOpType.add)
                    # after each s-half completes, run qkT for that half
                    if i == 3 or i == 7:
                        half = i // 4
                        for m in range(4):  # 0,1 = q tiles; 2,3 = k tiles
                            dstT = qT[m] if m < 2 else kT[m - 2]
                            pq = ps1.tile([128, 512], F32, tag="pqk", name="pqk")
                            for k in range(2):
                                nc.tensor.matmul(
                                    pq[:],
                                    wqkT[k][:, 128 * m:128 * (m + 1)],
                                    xnT[k][:, 512 * half:512 * (half + 1)],
                                    start=(k == 0), stop=(k == 1))
                            if has_bqk:
                                nc.vector.tensor_scalar_add(
                                    dstT[:, 512 * half:512 * (half + 1)], pq[:],
                                    bqk[:, m:m + 1])
                            else:
                                nc.vector.tensor_copy(
                                    dstT[:, 512 * half:512 * (half + 1)], pq[:])

            # ================= Phase 2: attention main loop =================
            with tc.tile_pool(name="battn", bufs=6) as bp, \
                 tc.tile_pool(name="ptp", bufs=4) as ptp, \
                 tc.tile_pool(name="smal", bufs=4) as sm, \
                 tc.tile_pool(name="ps2", bufs=2, space="PSUM") as ps2:
                for p in range(4):
                    h0, h1 = 2 * p, 2 * p + 1
                    ctx = ps2.tile([128, S], F32, tag="ctx", name="ctx", bufs=1)
                    for tj in range(2):
                        bts = {}
                        for hh in (h0, h1):
                            bt = bp.tile([128, 4096], BF16, tag="bt", name="bt")
                            src_ap = expbT_d.ap()[hh, 512 * tj:512 * (tj + 1), :]
                            src_ap = src_ap.rearrange("(a p) s -> p a s", p=128)
                            nc.sync.dma_start(bt[:].rearrange("p (a s) -> p a s", a=4), src_ap)
                            bts[hh] = bt
                        for ti in range(4):
                            t = 4 * tj + ti
                            for hh in (h0, h1):
                                band = 32 * (hh % 4)
                                ktile = kT[p // 2]
                                qtile = qT[p // 2]
                                sc = ps2.tile([128, S], F32, tag="sc", name="sc", bufs=3)
                                for half in range(2):
                                    nc.tensor.matmul(
                                        sc[:, 512 * half:512 * (half + 1)],
                                        ktile[band:band + 32, 128 * t:128 * (t + 1)],
                                        qtile[band:band + 32, 512 * half:512 * (half + 1)],
                                        start=True, stop=True,
                                        tile_position=(band, 0))
                                eq = ptp.tile([128, S], BF16, tag="eq", name="eq", bufs=4)
                                nc.scalar.activation(
                                    eq[:], sc[:], mybir.ActivationFunctionType.Exp)
                                pt = ptp.tile([128, S], BF16, tag="pt", name="pt", bufs=4)
                                tt_eng = nc.vector if (hh % 2 == 0) else nc.gpsimd
                                tt_eng.tensor_tensor(
                                    pt[:], eq[:],
                                    bts[hh][:, 1024 * ti:1024 * (ti + 1)],
                                    mybir.AluOpType.mult)
                                crow = 64 * (hh % 2)
                                for half in range(2):
                                    nc.tensor.matmul(
                                        ctx[crow:crow + 64, 512 * half:512 * (half + 1)],
                                        vx[t][:, 64 * hh:64 * (hh + 1)],
                                        pt[:, 512 * half:512 * (half + 1)],
                                        start=(t == 0), stop=(t == NT - 1),
                                        tile_position=(0, crow))
                    # evacuate unnormalized ctx + broadcast denominator row
                    for hh in (h0, h1):
                        crow = 64 * (hh % 2)
                        band = 32 * (hh % 4)
                        nc.vector.tensor_copy(
                            ctxU[hh // 4][band:band + 32, :],
                            ctx[crow:crow + 32, :])
                        nc.vector.stream_shuffle(
                            Rden[hh // 4][band:band + 32, :],
                            ctx[crow + 32:crow + 64, :], [0] * 32)
                # one wide reciprocal per 4-head group, then normalize
                for k in range(2):
                    nc.vector.reciprocal(Rrec[k][:], Rden[k][:])
                    nc.vector.tensor_tensor(ctxT[k][:], ctxU[k][:], Rrec[k][:],
                                            mybir.AluOpType.mult)

            if _DBG:
                for k in range(2):
                    nc.sync.dma_start(dbg_r8a.ap()[4*k:4*k+4, :], Rden[k][0:4, :].rearrange("p s -> p s"))
                    nc.sync.dma_start(dbg_r8r.ap()[4*k:4*k+4, :], Rrec[k][0:4, :])
                    du = pp.tile([128, S], F32, tag=f"du{k}", name=f"du{k}")
                    nc.vector.tensor_copy(du[:], ctxU[k][:])
                    nc.sync.dma_start(dbg_ctxU.ap()[k], du[:])
                    dt_ = pp.tile([128, S], F32, tag=f"dt{k}", name=f"dt{k}")
                    nc.vector.tensor_copy(dt_[:], ctxT[k][:])
                    nc.sync.dma_start(dbg_ctxT.ap()[k], dt_[:])
                    nc.sync.dma_start(dbg_R.ap()[k], Rrec[k][:])

            # ============ Phase 3: out-proj, residual, LN2 ============
            with tc.tile_pool(name="work3", bufs=4) as wk3, \
                 tc.tile_pool(name="ps3", bufs=2, space="PSUM") as ps3:
                ybf = [pp.tile([128, D], BF16, tag=f"ybf{i}", name=f"ybf{i}") for i in range(NT)]
                for i in range(NT):
                    pa = ps3.tile([128, D], F32, tag="pattn", name="pattn")
                    for k in range(2):
                        nc.tensor.matmul(
                            pa[:],
                            ctxT[k][:, 128 * i:128 * (i + 1)],
                            woT[k][:],
                            start=(k == 0), stop=(k == 1))
                    ht = wk3.tile([128, D], F32, tag="ht", name="ht")
                    nc.vector.tensor_tensor(ht[:], pa[:], x_res[i][:],
                                            mybir.AluOpType.add)
                    if has_bo:
                        ht2 = wk3.tile([128, D], F32, tag="ht2", name="ht2")
                        nc.vector.tensor_tensor(ht2[:], ht[:], bob[:],
                                                mybir.AluOpType.add)
                        ht = ht2
                    stats = wk3.tile([128, 6], F32, tag="lnstats", name="lnstats")
                    nc.vector.bn_stats(stats[:], ht[:])
                    aggr = wk3.tile([128, 2], F32, tag="lnaggr", name="lnaggr")
                    nc.vector.bn_aggr(aggr[:], stats[:])
                    sd = wk3.tile([128, 1], F32, tag="lnsd", name="lnsd")
                    nc.scalar.activation(sd[:], aggr[:, 1:2],
                                         mybir.ActivationFunctionType.Sqrt, bias=eps_t[:, 0:1])
                    rs = wk3.tile([128, 1], F32, tag="lnrs", name="lnrs")
                    nc.vector.reciprocal(rs[:], sd[:])
                    nc.vector.tensor_scalar(
                        yn[i][:], ht[:], aggr[:, 0:1], rs[:],
                        mybir.AluOpType.subtract, mybir.AluOpType.mult)
                    nc.gpsimd.tensor_copy(ybf[i][:], yn[i][:])
                    if aff2:
                        tmp = wk3.tile([128, D], F32, tag="afftmp2", name="afftmp2")
                        nc.vector.tensor_tensor(tmp[:], yn[i][:], g2b[:],
                                                mybir.AluOpType.mult)
                        nc.vector.tensor_tensor(y_res[i][:], tmp[:], be2b[:],
                                                mybir.AluOpType.add)
                transpose_to(ybf, ynT, ps3)

            # ================= Phase 4: FFN + output =================
            with tc.tile_pool(name="work4", bufs=4) as wk4, \
                 tc.tile_pool(name="ps4", bufs=2, space="PSUM") as ps4:
                for m in range(8):
                    for half in range(2):
                        pf = ps4.tile([128, 512], F32, tag="pf1", name="pf1")
                        for k in range(2):
                            nc.tensor.matmul(
                                pf[:],
                                w1T[k][:, 128 * m:128 * (m + 1)],
                                ynT[k][:, 512 * half:512 * (half + 1)],
                                start=(k == 0), stop=(k == 1))
                        bias_arg = b1c[:, m:m + 1] if has_b1 else 0.0
                        nc.scalar.activation(
                            f1T[m][:, 512 * half:512 * (half + 1)], pf[:],
                            mybir.ActivationFunctionType.Relu, bias=bias_arg)
                for m in range(2):
                    for half in range(2):
                        pf2 = ps4.tile([128, 512], F32, tag="pf2", name="pf2")
                        for k in range(8):
                            nc.tensor.matmul(
                                pf2[:],
                                w2T[k][:, 128 * m:128 * (m + 1)],
                                f1T[k][:, 512 * half:512 * (half + 1)],
                                start=(k == 0), stop=(k == 7))
                        if has_b2:
                            nc.vector.tensor_scalar_add(
                                f2T[m][:, 512 * half:512 * (half + 1)], pf2[:],
                                b2c[:, m:m + 1])
                        else:
                            nc.vector.tensor_copy(
                                f2T[m][:, 512 * half:512 * (half + 1)], pf2[:])
                # transpose f2T back to natural + final residual + store
                for i in range(NT):
                    tpn = ps4.tile([128, D], BF16, tag="tpn", name="tpn")
                    for j in range(2):
                        nc.tensor.transpose(
                            tpn[:, 128 * j:128 * (j + 1)],
                            f2T[j][:, 128 * i:128 * (i + 1)],
                            identb[:])
                    ot = wk4.tile([128, D], F32, tag="ot", name="ot")
                    nc.vector.tensor_tensor(ot[:], tpn[:], y_res[i][:],
                                            mybir.AluOpType.add)
                    nc.sync.dma_start(out_d.ap()[128 * i:128 * (i + 1), :], ot[:])

    nc.compile()
    return nc


def _prep_host(src, bias, in_proj_w, in_proj_b, out_w, out_b,
               w1, b1, w2, b2, g1, be1, g2, be2):
    f = np.float32
    g1 = np.asarray(g1, f); be1 = np.asarray(be1, f)
    g2 = np.asarray(g2, f); be2 = np.asarray(be2, f)
    in_proj_w = np.asarray(in_proj_w, f); in_proj_b = np.asarray(in_proj_b, f)
    out_w = np.asarray(out_w, f); out_b = np.asarray(out_b, f)
    w1 = np.asarray(w1, f); b1 = np.asarray(b1, f)
    w2 = np.asarray(w2, f); b2 = np.asarray(b2, f)

    winG = in_proj_w * g1[None, :]
    binG = in_proj_w @ be1 + in_proj_b
    scale = HD ** -0.5
    winG[0:D] *= scale
    binG[0:D] *= scale
    wqkT = np.ascontiguousarray(winG[0:2 * D].T).astype(bf16)      # [D, 2D]
    bqk = binG[0:2 * D]                                            # [2D]
    wv = winG[2 * D:3 * D]; bv = binG[2 * D:3 * D]
    # v_ext: head h occupies columns 64h..64h+63: [V_h (32) | ones (1) | 0...]
    wvxT = np.zeros((D, 2 * D), f)
    bvx = np.zeros((2 * D,), f)
    for h in range(H):
        wvxT[:, 64 * h:64 * h + 32] = wv[32 * h:32 * h + 32].T
        bvx[64 * h:64 * h + 32] = bv[32 * h:32 * h + 32]
        bvx[64 * h + 32] = 1.0
    w1G = w1 * g2[None, :]
    b1p = w1 @ be2 + b1

    flags = (
        bool(np.any(g1 != 1.0) or np.any(be1 != 0.0)),
        bool(np.any(g2 != 1.0) or np.any(be2 != 0.0)),
        bool(np.any(bqk != 0.0)),
        bool(np.any(out_b != 0.0)),
        bool(np.any(b1p != 0.0)),
        bool(np.any(b2 != 0.0)),
    )
    aff1, aff2, has_bqk, has_bo, has_b1, has_b2 = flags

    common = {
        "identb": np.eye(128, dtype=f).astype(bf16),
        "wqkT": wqkT,
        "wvxT": wvxT.astype(bf16),
        "bvx": np.broadcast_to(bvx, (128, 2 * D)).copy(),
        "woT": np.ascontiguousarray(out_w.T).astype(bf16),
        "w1T": np.ascontiguousarray(w1G.T).astype(bf16),
        "w2T": np.ascontiguousarray(w2.T).astype(bf16),
    }
    if has_bqk:
        common["bqk"] = np.ascontiguousarray(bqk.reshape(4, 128).T)
    if aff1:
        common["g1b"] = np.broadcast_to(g1, (128, D)).copy()
        common["be1b"] = np.broadcast_to(be1, (128, D)).copy()
    if aff2:
        common["g2b"] = np.broadcast_to(g2, (128, D)).copy()
        common["be2b"] = np.broadcast_to(be2, (128, D)).copy()
    if has_bo:
        common["bob"] = np.broadcast_to(out_b, (128, D)).copy()
    if has_b1:
        common["b1c"] = np.ascontiguousarray(b1p.reshape(DFF // 128, 128).T)
    if has_b2:
        common["b2c"] = np.ascontiguousarray(b2.reshape(D // 128, 128).T)

    src = np.asarray(src, f)
    bias = np.asarray(bias, f)
    # host: exp(bias) transposed -> bf16, per-core [H, S(t), S(s)]
    expbT = np.exp(bias.transpose(0, 1, 3, 2)).astype(bf16)
    expbT = np.ascontiguousarray(expbT)
    in_maps = []
    for b in range(N_CORES):
        m = dict(common)
        m["src"] = np.ascontiguousarray(src[:, b, :])
        m["expbT"] = expbT[b]
        in_maps.append(m)
    return flags, in_maps


def kernel(**inputs):
    _install_axon_hooks_shim()
    flags, in_maps = _prep_host(
        inputs["src"], inputs["bias"], inputs["in_proj_w"], inputs["in_proj_b"],
        inputs["out_w"], inputs["out_b"], inputs["w1"], inputs["b1"],
        inputs["w2"], inputs["b2"], inputs["g1"], inputs["be1"],
        inputs["g2"], inputs["be2"])
    if flags not in _CACHE:
        _CACHE[flags] = _build(flags)
    nc = _CACHE[flags]
    res = run_bass_kernel_spmd(nc, in_maps, core_ids=list(range(N_CORES)))
    out = np.empty((S, B, D), np.float32)
    for b in range(N_CORES):
        out[:, b, :] = res.results[b]["out"]
    return out



# revision 7
# speedup vs baseline: 1.3057x; 1.3057x over previous
"""Trainium2 Bass kernel for a biased transformer encoder layer.

Full (unsharded) inputs -> full output. Data-parallel over batch B across 8
NeuronCores (one batch element per core).

Key design points vs the naive lowering:
- The 256MB bias tensor is host-transposed to "scores^T" layout [t, s],
  pair-interleaved, cast to bf16, and *injected into PSUM by the tensor
  engine* (identity matmul, start=True) so the QK^T matmuls accumulate on
  top of it.  softmax's exp then runs as a single ScalarE activation per
  [128,1024] tile with no separate bias add/mult on the vector engine.
- Attention processes head PAIRS: the two heads' QK^T matmuls are row-tiled
  (bands b, b+32) and run concurrently in the PE array; the two heads' PV
  matmuls are col-tiled (output partitions 0-63 / 64-127) and also run
  concurrently.
- The softmax denominator comes from 32 replicated all-ones columns in the
  extended V operand, so the PV matmul broadcasts sum(p) across 32
  partitions for free (no stream_shuffle).
- All weights are host-packed into one [128, 6784] bf16 tile (single DMA);
  src / output use a single packed DMA each.
"""

import numpy as np
import ml_dtypes

import concourse.mybir as mybir
import concourse.tile as tile
from concourse import bacc
from concourse.bass_utils import run_bass_kernel_spmd

# ---- problem constants (hardcoded per contract) ----
S = 1024
B = 8
D = 256
H = 8
HD = D // H          # 32
DFF = 1024
EPS = 1e-5
N_CORES = 8
NT = S // 128        # 8 s-tiles / t-tiles

F32 = mybir.dt.float32
BF16 = mybir.dt.bfloat16
bf16 = ml_dtypes.bfloat16

# weight-pack column offsets (shared by _build and _prep_host)
OFF_ID = 0                      # identity          [128, 128]
OFF_QK = OFF_ID + 128           # wqkT  2x[128,512]
OFF_WV = OFF_QK + 1024          # wv dense 2x[128,256]
OFF_WO = OFF_WV + 512           # woPK  4x[128,256] (zero-padded pair blocks)
OFF_W1 = OFF_WO + 1024          # w1T   2x[128,1024]
OFF_W2 = OFF_W1 + 2048          # w2T   8x[128,256]
WPK_COLS = OFF_W2 + 2048        # 6784

_CACHE = {}


def _install_axon_hooks_shim():
    """Make `trace=True` degrade gracefully if antenv.axon_hooks is missing."""
    import sys, types
    try:
        import antenv  # noqa
    except ImportError:
        return
    if "antenv.axon_hooks" in sys.modules:
        return
    try:
        import antenv.axon_hooks  # noqa
    except ImportError:
        import antenv
        mod = types.ModuleType("antenv.axon_hooks")
        _hook = [None]
        mod.set_axon_ntff_profile_hook = lambda h: _hook.__setitem__(0, h)
        mod.get_axon_ntff_profile_hook = lambda: _hook[0]
        sys.modules["antenv.axon_hooks"] = mod
        antenv.axon_hooks = mod


def _build(flags):
    """Build the Bass program (shared by all 8 cores, SPMD)."""
    (aff1, aff2, has_bqk, has_bo, has_b1, has_b2, has_bv) = flags
    nc = bacc.Bacc("TRN2", debug=False, num_devices=N_CORES, enable_asserts=True)

    # ---- DRAM tensors (per-core inputs) ----
    src_d = nc.dram_tensor("src", [S, D], F32, kind="ExternalInput")
    # raw bias^T, bf16, packed [pair, tpair, trow, 4096]
    # cols: t-even / t-odd halves of 2048; within each: [h0 s0:512 | h1 s0:512
    #       | h0 s512:1024 | h1 s512:1024]
    expb_d = nc.dram_tensor("biasPK", [4, 4, 128, 4096], BF16, kind="ExternalInput")
    wpk_d = nc.dram_tensor("wpk", [128, WPK_COLS], BF16, kind="ExternalInput")
    if has_bv:
        bvb_d = nc.dram_tensor("bvb", [128, D], F32, kind="ExternalInput")
    if has_bqk:
        bqk_d = nc.dram_tensor("bqk", [128, 4], F32, kind="ExternalInput")
    if aff1:
        g1b_d = nc.dram_tensor("g1b", [128, D], F32, kind="ExternalInput")
        be1b_d = nc.dram_tensor("be1b", [128, D], F32, kind="ExternalInput")
    if aff2:
        g2b_d = nc.dram_tensor("g2b", [128, D], F32, kind="ExternalInput")
        be2b_d = nc.dram_tensor("be2b", [128, D], F32, kind="ExternalInput")
    if has_bo:
        bob_d = nc.dram_tensor("bob", [128, D], F32, kind="ExternalInput")
    if has_b1:
        b1c_d = nc.dram_tensor("b1c", [128, DFF // 128], F32, kind="ExternalInput")
    if has_b2:
        b2c_d = nc.dram_tensor("b2c", [128, D // 128], F32, kind="ExternalInput")
    out_d = nc.dram_tensor("out", [S, D], F32, kind="ExternalOutput")

    with tile.TileContext(nc, trace_sim=True) as tc:
        with tc.tile_pool(name="persist", bufs=1) as pp:
            # ---- packed weights, one DMA ----
            wpk = pp.tile([128, WPK_COLS], BF16, tag="wpk", name="wpk")
            nc.sync.dma_start(wpk[:], wpk_d.ap())
            identb = wpk[:, OFF_ID:OFF_ID + 128]
            eps_t = pp.tile([128, 1], F32, tag="eps_t", name="eps_t")
            nc.gpsimd.memset(eps_t[:], EPS)
            if has_bv:
                bvb = pp.tile([128, D], F32, tag="bvb", name="bvb")
                nc.sync.dma_start(bvb[:], bvb_d.ap())
            if has_bqk:
                bqk = pp.tile([128, 4], F32, tag="bqk", name="bqk")
                nc.sync.dma_start(bqk[:], bqk_d.ap())
            if aff1:
                g1b = pp.tile([128, D], F32, tag="g1b", name="g1b")
                be1b = pp.tile([128, D], F32, tag="be1b", name="be1b")
                nc.sync.dma_start(g1b[:], g1b_d.ap())
                nc.sync.dma_start(be1b[:], be1b_d.ap())
            if aff2:
                g2b = pp.tile([128, D], F32, tag="g2b", name="g2b")
                be2b = pp.tile([128, D], F32, tag="be2b", name="be2b")
                nc.sync.dma_start(g2b[:], g2b_d.ap())
                nc.sync.dma_start(be2b[:], be2b_d.ap())
            if has_bo:
                bob = pp.tile([128, D], F32, tag="bob", name="bob")
                nc.sync.dma_start(bob[:], bob_d.ap())
            if has_b1:
                b1c = pp.tile([128, DFF // 128], F32, tag="b1c", name="b1c")
                nc.sync.dma_start(b1c[:], b1c_d.ap())
            if has_b2:
                b2c = pp.tile([128, D // 128], F32, tag="b2c", name="b2c")
                nc.sync.dma_start(b2c[:], b2c_d.ap())

            # ---- persistent activations ----
            srcall = pp.tile([128, 8 * D], F32, tag="srcall", name="srcall")
            xbf = [pp.tile([128, D], BF16, tag=f"xbf{i}", name=f"xbf{i}") for i in range(NT)]
            x_res = xbf
            if aff1:
                xn = [pp.tile([128, D], F32, tag=f"xn{i}", name=f"xn{i}") for i in range(NT)]
                x_res = [pp.tile([128, D], F32, tag=f"xr{i}", name=f"xr{i}") for i in range(NT)]
            xnT = [pp.tile([128, S], BF16, tag=f"xnT{k}", name=f"xnT{k}") for k in range(2)]
            qT = [pp.tile([128, S], BF16, tag=f"qT{k}", name=f"qT{k}") for k in range(2)]
            kT = [pp.tile([128, S], BF16, tag=f"kT{k}", name=f"kT{k}") for k in range(2)]
            vx = [pp.tile([128, 2 * D], BF16, tag=f"vx{i}", name=f"vx{i}") for i in range(NT)]
            ctxN = [pp.tile([128, S], BF16, tag=f"ctxN{p}", name=f"ctxN{p}") for p in range(4)]
            ybf = [pp.tile([128, D], BF16, tag=f"ybf{i}", name=f"ybf{i}") for i in range(NT)]
            y_res = ybf
            if aff2:
                yn = [pp.tile([128, D], F32, tag=f"yn{i}", name=f"yn{i}") for i in range(NT)]
                y_res = [pp.tile([128, D], F32, tag=f"yr{i}", name=f"yr{i}") for i in range(NT)]
            ynT = [pp.tile([128, S], BF16, tag=f"ynT{k}", name=f"ynT{k}") for k in range(2)]
            f1T = [pp.tile([128, S], BF16, tag=f"f1T{m}", name=f"f1T{m}") for m in range(8)]
            f2T = [pp.tile([128, S], BF16, tag=f"f2T{m}", name=f"f2T{m}") for m in range(2)]
            outbuf = pp.tile([128, 8 * D], F32, tag="outbuf", name="outbuf")

            # vx tiles: memset to 1.0; v columns overwritten later.  The
            # surviving 1.0 columns are the denominator "ones" blocks.
            for i in range(NT):
                nc.gpsimd.memset(vx[i][:], 1.0)

            def ln_to(st, dst_bf, work, xn_f32=None):
                """LayerNorm rows of st -> bf16 dst (and optional f32 copy)."""
                stats = work.tile([128, 6], F32, tag="lnstats", name="lnstats")
                nc.vector.bn_stats(stats[:], st)
                aggr = work.tile([128, 2], F32, tag="lnaggr", name="lnaggr")
                nc.vector.bn_aggr(aggr[:], stats[:])
                sd = work.tile([128, 1], F32, tag="lnsd", name="lnsd")
                nc.scalar.activation(sd[:], aggr[:, 1:2],
                                     mybir.ActivationFunctionType.Sqrt,
                                     bias=eps_t[:, 0:1])
                rs = work.tile([128, 1], F32, tag="lnrs", name="lnrs")
                nc.vector.reciprocal(rs[:], sd[:])
                if xn_f32 is not None:
                    nc.vector.tensor_scalar(
                        xn_f32, st, aggr[:, 0:1], rs[:],
                        mybir.AluOpType.subtract, mybir.AluOpType.mult)
                    nc.gpsimd.tensor_copy(dst_bf, xn_f32)
                else:
                    nc.vector.tensor_scalar(
                        dst_bf, st, aggr[:, 0:1], rs[:],
                        mybir.AluOpType.subtract, mybir.AluOpType.mult)

            # ================= Phase 1: LN1, transposes, v, qkT =============
            with tc.tile_pool(name="work1", bufs=4) as wk, \
                 tc.tile_pool(name="ps1", bufs=2, space="PSUM") as ps1:
                nc.sync.dma_start(
                    srcall[:].rearrange("p (i d) -> p i d", i=NT),
                    src_d.ap().rearrange("(i p) d -> p i d", p=128))
                for i in range(NT):
                    st = srcall[:, D * i:D * (i + 1)]
                    ln_to(st, xbf[i][:], wk, xn_f32=(xn[i][:] if aff1 else None))
                    if aff1:
                        tmp = wk.tile([128, D], F32, tag="afftmp", name="afftmp")
                        nc.vector.tensor_tensor(tmp[:], xn[i][:], g1b[:],
                                                mybir.AluOpType.mult)
                        nc.vector.tensor_tensor(x_res[i][:], tmp[:], be1b[:],
                                                mybir.AluOpType.add)
                    # transpose s-tile into xnT columns (both d-blocks)
                    tp = ps1.tile([128, 256], BF16, tag="tp", name="tp")
                    for j in range(2):
                        nc.tensor.transpose(
                            tp[:, 128 * j:128 * (j + 1)],
                            xbf[i][:, 128 * j:128 * (j + 1)], identb)
                    for j in range(2):
                        nc.vector.tensor_copy(
                            xnT[j][:, 128 * i:128 * (i + 1)],
                            tp[:, 128 * j:128 * (j + 1)])
                    # dense v projection for this tile
                    pv = ps1.tile([128, D], F32, tag="pv", name="pv")
                    for k in range(2):
                        nc.tensor.matmul(
                            pv[:],
                            xnT[k][:, 128 * i:128 * (i + 1)],
                            wpk[:, OFF_WV + 256 * k:OFF_WV + 256 * (k + 1)],
                            start=(k == 0), stop=(k == 1))
                    vdst = vx[i][:].rearrange("p (h c) -> p h c", h=H)[:, :, 0:HD]
                    vsrc = pv[:].rearrange("p (h c) -> p h c", h=H)
                    if has_bv:
                        bsrc = bvb[:].rearrange("p (h c) -> p h c", h=H)
                        nc.vector.tensor_tensor(vdst, vsrc, bsrc,
                                                mybir.AluOpType.add)
                    else:
                        nc.vector.tensor_copy(vdst, vsrc)
                    # qkT for each completed s-half
                    if i == 3 or i == 7:
                        half = i // 4
                        for m in range(4):  # 0,1 = q tiles; 2,3 = k tiles
                            dstT = qT[m] if m < 2 else kT[m - 2]
                            pq = ps1.tile([128, 512], F32, tag="pq", name="pq")
                            for k in range(2):
                                nc.tensor.matmul(
                                    pq[:],
                                    wpk[:, OFF_QK + 512 * k + 128 * m:
                                        OFF_QK + 512 * k + 128 * (m + 1)],
                                    xnT[k][:, 512 * half:512 * (half + 1)],
                                    start=(k == 0), stop=(k == 1))
                            if has_bqk:
                                nc.vector.tensor_scalar_add(
                                    dstT[:, 512 * half:512 * (half + 1)], pq[:],
                                    bqk[:, m:m + 1])
                            else:
                                nc.vector.tensor_copy(
                                    dstT[:, 512 * half:512 * (half + 1)], pq[:])

            # ================= Phase 2: attention =================
            with tc.tile_pool(name="battn", bufs=3) as bp, \
                 tc.tile_pool(name="ptp", bufs=3) as ptp, \
                 tc.tile_pool(name="evac", bufs=2) as ev, \
                 tc.tile_pool(name="ps2", bufs=1, space="PSUM") as ps2:
                ebs = {}
                for p in range(4):
                    g = p // 2
                    b0 = 32 * ((2 * p) % 4)
                    b1 = b0 + 32
                    h0, h1 = 2 * p, 2 * p + 1
                    ctx = ps2.tile([128, S], F32, tag="ctx", name="ctx", bufs=1)
                    for t in range(NT):
                        tj = t // 2
                        if t % 2 == 0:
                            eb = bp.tile([128, 4096], BF16, tag="eb", name="eb")
                            nc.sync.dma_start(eb[:], expb_d.ap()[p, tj])
                            ebs[(p, tj)] = eb
                        eb = ebs[(p, tj)]
                        off = 2048 * (t % 2)
                        scs = [ps2.tile([128, 1024], F32, tag="sc", name="sc",
                                        bufs=3) for _ in range(2)]
                        # inject raw bias into PSUM (sets has_written)
                        for half in range(2):
                            for hh in range(2):
                                nc.tensor.matmul(
                                    scs[half][:, 512 * hh:512 * (hh + 1)],
                                    identb,
                                    eb[:, off + 1024 * half + 512 * hh:
                                        off + 1024 * half + 512 * (hh + 1)],
                                    start=True, stop=False)
                        # QK^T accumulates on top; band-pairs run concurrent
                        for bnd, hh in ((b0, 0), (b1, 1)):
                            for half in range(2):
                                nc.tensor.matmul(
                                    scs[half][:, 512 * hh:512 * (hh + 1)],
                                    kT[g][bnd:bnd + 32, 128 * t:128 * (t + 1)],
                                    qT[g][bnd:bnd + 32,
                                          512 * half:512 * (half + 1)],
                                    start=False, stop=True,
                                    tile_position=(bnd, 0))
                        pts = []
                        for half in range(2):
                            pt = ptp.tile([128, 1024], BF16, tag="pt", name="pt")
                            nc.scalar.activation(
                                pt[:], scs[half][:],
                                mybir.ActivationFunctionType.Exp)
                            pts.append(pt)
                        st0, sp0 = (t == 0), (t == NT - 1)
                        for half in range(2):
                            pt = pts[half]
                            nc.tensor.matmul(
                                ctx[0:64, 512 * half:512 * (half + 1)],
                                vx[t][:, 64 * h0:64 * (h0 + 1)],
                                pt[:, 0:512],
                                start=st0, stop=sp0, tile_position=(0, 0))
                            nc.tensor.matmul(
                                ctx[64:128, 512 * half:512 * (half + 1)],
                                vx[t][:, 64 * h1:64 * (h1 + 1)],
                                pt[:, 512:1024],
                                start=st0, stop=sp0, tile_position=(0, 64))
                    # ---- evacuate + normalize this pair ----
                    U = ev.tile([128, S], BF16, tag="U", name="U")
                    nc.vector.tensor_copy(U[:], ctx[:])
                    R = ev.tile([128, S], BF16, tag="R", name="R")
                    with nc.allow_low_precision(
                            reason="softmax denom reciprocal in bf16; "
                                   "0.4% rel err is within tolerance"):
                        nc.vector.reciprocal(R[:], U[:])
                    Rb = ev.tile([128, S], BF16, tag="Rb", name="Rb")
                    nc.gpsimd.memset(Rb[32:64, :], 0.0)
                    nc.gpsimd.memset(Rb[96:128, :], 0.0)
                    nc.vector.tensor_copy(Rb[0:32, :], R[32:64, :])
                    nc.vector.tensor_copy(Rb[64:96, :], R[96:128, :])
                    nc.vector.tensor_tensor(ctxN[p][:], U[:], Rb[:],
                                            mybir.AluOpType.mult)

            # ============ Phase 3: out-proj, residual, LN2 ============
            with tc.tile_pool(name="work3", bufs=4) as wk3, \
                 tc.tile_pool(name="ps3", bufs=2, space="PSUM") as ps3:
                for i in range(NT):
                    pa = ps3.tile([128, D], F32, tag="pa", name="pa")
                    for p in range(4):
                        nc.tensor.matmul(
                            pa[:],
                            ctxN[p][:, 128 * i:128 * (i + 1)],
                            wpk[:, OFF_WO + 256 * p:OFF_WO + 256 * (p + 1)],
                            start=(p == 0), stop=(p == 3))
                    ht = wk3.tile([128, D], F32, tag="ht", name="ht")
                    nc.vector.tensor_tensor(ht[:], pa[:], x_res[i][:],
                                            mybir.AluOpType.add)
                    if has_bo:
                        ht2 = wk3.tile([128, D], F32, tag="ht2", name="ht2")
                        nc.vector.tensor_tensor(ht2[:], ht[:], bob[:],
                                                mybir.AluOpType.add)
                        ht = ht2
                    ln_to(ht[:], ybf[i][:], wk3,
                          xn_f32=(yn[i][:] if aff2 else None))
                    if aff2:
                        tmp = wk3.tile([128, D], F32, tag="afftmp2", name="afftmp2")
                        nc.vector.tensor_tensor(tmp[:], yn[i][:], g2b[:],
                                                mybir.AluOpType.mult)
                        nc.vector.tensor_tensor(y_res[i][:], tmp[:], be2b[:],
                                                mybir.AluOpType.add)
                # transpose ybf -> ynT (batched 4 tiles per PSUM tile)
                for j in range(2):
                    for i0 in range(0, NT, 4):
                        tpb = ps3.tile([128, 512], BF16, tag="tpb", name="tpb")
                        for di in range(4):
                            i = i0 + di
                            nc.tensor.transpose(
                                tpb[:, 128 * di:128 * (di + 1)],
                                ybf[i][:, 128 * j:128 * (j + 1)], identb)
                        nc.vector.tensor_copy(
                            ynT[j][:, 128 * i0:128 * (i0 + 4)], tpb[:])

            # ================= Phase 4: FFN + output =================
            with tc.tile_pool(name="ps4", bufs=2, space="PSUM") as ps4:
                for m in range(8):
                    for half in range(2):
                        pf = ps4.tile([128, 512], F32, tag="pf1", name="pf1",
                                      bufs=3)
                        for k in range(2):
                            nc.tensor.matmul(
                                pf[:],
                                wpk[:, OFF_W1 + 1024 * k + 128 * m:
                                    OFF_W1 + 1024 * k + 128 * (m + 1)],
                                ynT[k][:, 512 * half:512 * (half + 1)],
                                start=(k == 0), stop=(k == 1))
                        if has_b1:
                            nc.scalar.activation(
                                f1T[m][:, 512 * half:512 * (half + 1)], pf[:],
                                mybir.ActivationFunctionType.Relu,
                                bias=b1c[:, m:m + 1])
                        else:
                            nc.vector.tensor_scalar_max(
                                f1T[m][:, 512 * half:512 * (half + 1)], pf[:],
                                0.0)
                for m in range(2):
                    for half in range(2):
                        pf2 = ps4.tile([128, 512], F32, tag="pf2", name="pf2")
                        for k in range(8):
                            nc.tensor.matmul(
                                pf2[:],
                                wpk[:, OFF_W2 + 256 * k + 128 * m:
                                    OFF_W2 + 256 * k + 128 * (m + 1)],
                                f1T[k][:, 512 * half:512 * (half + 1)],
                                start=(k == 0), stop=(k == 7))
                        if has_b2:
                            nc.vector.tensor_scalar_add(
                                f2T[m][:, 512 * half:512 * (half + 1)], pf2[:],
                                b2c[:, m:m + 1])
                        else:
                            nc.vector.tensor_copy(
                                f2T[m][:, 512 * half:512 * (half + 1)], pf2[:])
                # transpose back + final residual into packed out buffer
                for i in range(NT):
                    tpn = ps4.tile([128, D], BF16, tag="tpn", name="tpn")
                    for j in range(2):
                        nc.tensor.transpose(
                            tpn[:, 128 * j:128 * (j + 1)],
                            f2T[j][:, 128 * i:128 * (i + 1)], identb)
                    nc.vector.tensor_tensor(
                        outbuf[:, D * i:D * (i + 1)], tpn[:], y_res[i][:],
                        mybir.AluOpType.add)
                nc.sync.dma_start(
                    out_d.ap().rearrange("(i p) d -> p i d", p=128),
                    outbuf[:].rearrange("p (i d) -> p i d", i=NT))

    nc.compile()
    return nc


def _prep_host(src, bias, in_proj_w, in_proj_b, out_w, out_b,
               w1, b1, w2, b2, g1, be1, g2, be2):
    f = np.float32
    g1 = np.asarray(g1, f); be1 = np.asarray(be1, f)
    g2 = np.asarray(g2, f); be2 = np.asarray(be2, f)
    in_proj_w = np.asarray(in_proj_w, f); in_proj_b = np.asarray(in_proj_b, f)
    out_w = np.asarray(out_w, f); out_b = np.asarray(out_b, f)
    w1 = np.asarray(w1, f); b1 = np.asarray(b1, f)
    w2 = np.asarray(w2, f); b2 = np.asarray(b2, f)

    winG = in_proj_w * g1[None, :]
    binG = in_proj_w @ be1 + in_proj_b
    scale = HD ** -0.5
    winG[0:D] *= scale
    binG[0:D] *= scale
    wqkT = np.ascontiguousarray(winG[0:2 * D].T)               # [D, 2D]
    bqk = binG[0:2 * D]                                        # [2D]
    wv = winG[2 * D:3 * D]                                     # [D_v, D]
    bv = binG[2 * D:3 * D]
    wvd = np.ascontiguousarray(wv.T)                           # [D, D] dense
    w1G = w1 * g2[None, :]
    b1p = w1 @ be2 + b1

    flags = (
        bool(np.any(g1 != 1.0) or np.any(be1 != 0.0)),
        bool(np.any(g2 != 1.0) or np.any(be2 != 0.0)),
        bool(np.any(bqk != 0.0)),
        bool(np.any(out_b != 0.0)),
        bool(np.any(b1p != 0.0)),
        bool(np.any(b2 != 0.0)),
        bool(np.any(bv != 0.0)),
    )
    aff1, aff2, has_bqk, has_bo, has_b1, has_b2, has_bv = flags

    # ---- weight pack ----
    wpk = np.zeros((128, WPK_COLS), f)
    wpk[:, OFF_ID:OFF_ID + 128] = np.eye(128, dtype=f)
    for k in range(2):
        wpk[:, OFF_QK + 512 * k:OFF_QK + 512 * (k + 1)] = wqkT[128 * k:128 * (k + 1)]
        wpk[:, OFF_WV + 256 * k:OFF_WV + 256 * (k + 1)] = wvd[128 * k:128 * (k + 1)]
        wpk[:, OFF_W1 + 1024 * k:OFF_W1 + 1024 * (k + 1)] = \
            np.ascontiguousarray(w1G.T)[128 * k:128 * (k + 1)]
    woT = np.ascontiguousarray(out_w.T)                        # [D, D]
    for p in range(4):
        blk = np.zeros((128, D), f)
        blk[0:32] = woT[64 * p:64 * p + 32]        # head 2p
        blk[64:96] = woT[64 * p + 32:64 * p + 64]  # head 2p+1
        wpk[:, OFF_WO + 256 * p:OFF_WO + 256 * (p + 1)] = blk
    w2T = np.ascontiguousarray(w2.T)                           # [DFF, D]
    for k in range(8):
        wpk[:, OFF_W2 + 256 * k:OFF_W2 + 256 * (k + 1)] = w2T[128 * k:128 * (k + 1)]

    # ---- bias pack: raw bias^T bf16, pair-interleaved ----
    src = np.asarray(src, f)
    bias = np.asarray(bias, f)
    bT = bias.transpose(0, 1, 3, 2)                # [B, H, t, s]
    #  [B, p, i, tj, u, r, half, s512] -> [B, p, tj, r, u, half, i, s512]
    x = bT.reshape(B, 4, 2, 4, 2, 128, 2, 512)
    x = x.transpose(0, 1, 3, 5, 4, 6, 2, 7)
    biasPK = np.ascontiguousarray(x.reshape(B, 4, 4, 128, 4096)).astype(bf16)

    common = {"wpk": wpk.astype(bf16)}
    if has_bv:
        bvbt = np.zeros((128, D), f)
        for h in range(H):
            bvbt[:, 32 * h:32 * (h + 1)] = bv[32 * h:32 * (h + 1)]
        common["bvb"] = bvbt
    if has_bqk:
        common["bqk"] = np.ascontiguousarray(bqk.reshape(4, 128).T)
    if aff1:
        common["g1b"] = np.broadcast_to(g1, (128, D)).copy()
        common["be1b"] = np.broadcast_to(be1, (128, D)).copy()
    if aff2:
        common["g2b"] = np.broadcast_to(g2, (128, D)).copy()
        common["be2b"] = np.broadcast_to(be2, (128, D)).copy()
    if has_bo:
        common["bob"] = np.broadcast_to(out_b, (128, D)).copy()
    if has_b1:
        common["b1c"] = np.ascontiguousarray(b1p.reshape(DFF // 128, 128).T)
    if has_b2:
        common["b2c"] = np.ascontiguousarray(b2.reshape(D // 128, 128).T)

    in_maps = []
    for b in range(N_CORES):
        m = dict(common)
        m["src"] = np.ascontiguousarray(src[:, b, :])
        m["biasPK"] = biasPK[b]
        in_maps.append(m)
    return flags, in_maps


def kernel(**inputs):
    _install_axon_hooks_shim()
    flags, in_maps = _prep_host(
        inputs["src"], inputs["bias"], inputs["in_proj_w"], inputs["in_proj_b"],
        inputs["out_w"], inputs["out_b"], inputs["w1"], inputs["b1"],
        inputs["w2"], inputs["b2"], inputs["g1"], inputs["be1"],
        inputs["g2"], inputs["be2"])
    if flags not in _CACHE:
        _CACHE[flags] = _build(flags)
    nc = _CACHE[flags]
    res = run_bass_kernel_spmd(nc, in_maps, core_ids=list(range(N_CORES)))
    out = np.empty((S, B, D), np.float32)
    for b in range(N_CORES):
        out[:, b, :] = res.results[b]["out"]
    return out
